# revision 6
# baseline (speedup 1.0000x reference)
"""GridMask kernel for Trainium2, 8-core data parallel.

out[b,h,w,c] = x[b,h,w,c] * row_keep[b,h] * col_keep[b,w]

The grid mask is separable: a pixel survives iff its row is outside the
horizontal stripes AND its column is outside the vertical stripes. The
tiny per-image row/col keep vectors are computed host-side with exact
integer math; the device kernel streams the 100 MB image tensor through
SBUF applying both mask factors in a single fused
scalar_tensor_tensor op per 128-row tile.

Per core: 4 images, each image split into 4 tiles of [128 rows, 1536
floats] (row-major H x (W*C)). Row mask enters as a per-partition
scalar, col mask as a partition-broadcast [128, 1536] tile loaded once
per image.
"""

import math

import numpy as np

import concourse.bass as bass
import concourse.mybir as mybir
from concourse import bacc, tile
from concourse.bass_utils import run_bass_kernel_spmd

B, H, W, C = 32, 512, 512, 3
D1 = 96
HH = math.ceil(math.sqrt(H * H + W * W))  # 725
OFF_H = (HH - H) // 2  # 106
OFF_W = (HH - W) // 2  # 106

NCORES = 8
BPC = B // NCORES  # images per core
FREE = W * C  # 1536 floats per image row
NBLK = H // 128  # row blocks per image
ROWS = BPC * H  # SBUF-tile rows per core slab

F32 = mybir.dt.float32

_CACHE: dict = {}


def _build_masks(d_raw, st_h_raw, st_w_raw):
    """Exact replica of the reference's integer mask math, in numpy."""
    d = D1 + d_raw.astype(np.int64)  # [B] stripe period
    l = (d + 1) // 2  # ceil(d * 0.5) for integer d
    st_h = st_h_raw.astype(np.int64) % d
    st_w = st_w_raw.astype(np.int64) % d
    yy = OFF_H + np.arange(H, dtype=np.int64)
    xx = OFF_W + np.arange(W, dtype=np.int64)
    row_zero = ((yy[None, :] - st_h[:, None]) % d[:, None]) < l[:, None]
    col_zero = ((xx[None, :] - st_w[:, None]) % d[:, None]) < l[:, None]
    row_keep = (~row_zero).astype(np.float32)  # [B,H]
    col_keep = (~col_zero).astype(np.float32)  # [B,W]
    return row_keep, col_keep


def _build_nc():
    nc = bacc.Bacc(None)
    x = nc.dram_tensor("x", [ROWS, FREE], F32, kind="ExternalInput")
    rowm = nc.dram_tensor("rowm", [128, BPC * NBLK], F32, kind="ExternalInput")
    colm = nc.dram_tensor("colm", [BPC * FREE], F32, kind="ExternalInput")
    y = nc.dram_tensor("y", [ROWS, FREE], F32, kind="ExternalOutput")

    mult = mybir.AluOpType.mult
    with tile.TileContext(nc) as tc:
        with (
            tc.tile_pool(name="const", bufs=1) as cpool,
            tc.tile_pool(name="io", bufs=4) as iop,
        ):
            rowm_sb = cpool.tile([128, BPC * NBLK], F32, tag="rowm")
            nc.sync.dma_start(rowm_sb[:], rowm[:])
            colm_sb = cpool.tile([128, BPC * FREE], F32, tag="colm")
            nc.sync.dma_start(colm_sb[:], colm[:].partition_broadcast(128))
            # Fence: absorb the mask-DMA waits into VectorE program order so
            # each scalar_tensor_tensor below only waits on its own x load.
            scratch = cpool.tile([128, 2], F32, tag="scratch")
            nc.vector.tensor_copy(scratch[:, 0:1], rowm_sb[:, 0:1])
            nc.vector.tensor_copy(scratch[:, 1:2], colm_sb[:, 0:1])
            for img in range(BPC):
                for blk in range(NBLK):
                    t = img * NBLK + blk
                    r0 = t * 128
                    xt = iop.tile([128, FREE], F32, tag="xt")
                    nc.sync.dma_start(xt[:], x[r0 : r0 + 128, :])
                    yt = iop.tile([128, FREE], F32, tag="yt")
                    nc.vector.scalar_tensor_tensor(
                        yt[:],
                        xt[:],
                        rowm_sb[:, t : t + 1],
                        colm_sb[:, img * FREE : (img + 1) * FREE],
                        op0=mult,
                        op1=mult,
                    )
                    nc.sync.dma_start(y[r0 : r0 + 128, :], yt[:])
    nc.compile()
    return nc


def _prep_inputs(x, d_raw, st_h_raw, st_w_raw):
    x = np.ascontiguousarray(np.asarray(x, dtype=np.float32))
    row_keep, col_keep = _build_masks(
        np.asarray(d_raw), np.asarray(st_h_raw), np.asarray(st_w_raw)
    )
    col_exp = np.repeat(col_keep, C, axis=1)  # [B, W*C]
    in_maps = []
    for c in range(NCORES):
        sl = slice(c * BPC, (c + 1) * BPC)
        xc = x[sl].reshape(ROWS, FREE)
        # rowm[p, img*NBLK+blk] = row_keep[img, blk*128+p]
        rm = np.ascontiguousarray(
            row_keep[sl].reshape(BPC * NBLK, 128).T
        )
        cm = np.ascontiguousarray(col_exp[sl]).reshape(-1)
        in_maps.append({"x": xc, "rowm": rm, "colm": cm})
    return in_maps


def kernel(x, d_raw, st_h_raw, st_w_raw):
    if "nc" not in _CACHE:
        _CACHE["nc"] = _build_nc()
    nc = _CACHE["nc"]
    in_maps = _prep_inputs(x, d_raw, st_h_raw, st_w_raw)
    res = run_bass_kernel_spmd(nc, in_maps, list(range(NCORES)))
    out = np.concatenate(
        [r["y"].reshape(BPC, H, W, C) for r in res.results], axis=0
    )
    return out


# revision 9
# speedup vs baseline: 1.1027x; 1.1027x over previous
"""GridMask kernel for Trainium2, 8-core data parallel.

out[b,h,w,c] = x[b,h,w,c] * row_keep[b,h] * col_keep[b,w]

The grid mask is separable: a pixel survives iff its row is outside the
horizontal stripes AND its column is outside the vertical stripes. The
tiny per-image row/col keep vectors are computed host-side with exact
integer math; the device kernel streams the 100 MB image tensor through
SBUF applying both mask factors in a single fused
scalar_tensor_tensor op per 128-row tile.

Per core: 4 images, each image split into 4 tiles of [128 rows, 1536
floats] (row-major H x (W*C)). Row mask enters as a per-partition
scalar, col mask as a partition-broadcast [128, 1536] tile loaded once
per image.
"""

import math

import numpy as np

import concourse.bass as bass
import concourse.mybir as mybir
from concourse import bacc, tile
from concourse.bass_utils import run_bass_kernel_spmd

B, H, W, C = 32, 512, 512, 3
D1 = 96
HH = math.ceil(math.sqrt(H * H + W * W))  # 725
OFF_H = (HH - H) // 2  # 106
OFF_W = (HH - W) // 2  # 106

NCORES = 8
BPC = B // NCORES  # images per core
FREE = W * C  # 1536 floats per image row
NBLK = H // 128  # row blocks per image
ROWS = BPC * H  # SBUF-tile rows per core slab

F32 = mybir.dt.float32

_CACHE: dict = {}


def _build_masks(d_raw, st_h_raw, st_w_raw):
    """Exact replica of the reference's integer mask math, in numpy."""
    d = D1 + d_raw.astype(np.int64)  # [B] stripe period
    l = (d + 1) // 2  # ceil(d * 0.5) for integer d
    st_h = st_h_raw.astype(np.int64) % d
    st_w = st_w_raw.astype(np.int64) % d
    yy = OFF_H + np.arange(H, dtype=np.int64)
    xx = OFF_W + np.arange(W, dtype=np.int64)
    row_zero = ((yy[None, :] - st_h[:, None]) % d[:, None]) < l[:, None]
    col_zero = ((xx[None, :] - st_w[:, None]) % d[:, None]) < l[:, None]
    row_keep = (~row_zero).astype(np.float32)  # [B,H]
    col_keep = (~col_zero).astype(np.float32)  # [B,W]
    return row_keep, col_keep


ROWS_PER_PART = H // 128  # 4 consecutive image rows per partition
TILE_FREE = ROWS_PER_PART * FREE  # 6144 floats = 24 KB per partition


def _build_nc():
    nc = bacc.Bacc(None)
    # One tile per image: partition p holds image rows 4p..4p+3, which are
    # contiguous 24 KB in DRAM — big DMA packets instead of 6 KB row packets.
    x = nc.dram_tensor("x", [BPC, 128, TILE_FREE], F32, kind="ExternalInput")
    rowm = nc.dram_tensor("rowm", [128, BPC * ROWS_PER_PART], F32, kind="ExternalInput")
    colm = nc.dram_tensor("colm", [BPC * FREE], F32, kind="ExternalInput")
    y = nc.dram_tensor("y", [BPC, 128, TILE_FREE], F32, kind="ExternalOutput")

    mult = mybir.AluOpType.mult
    with tile.TileContext(nc) as tc:
        with (
            tc.tile_pool(name="const", bufs=1) as cpool,
            tc.tile_pool(name="io", bufs=2) as iop,
        ):
            rowm_sb = cpool.tile([128, BPC * ROWS_PER_PART], F32, tag="rowm")
            nc.sync.dma_start(rowm_sb[:], rowm[:])
            colm_sb = cpool.tile([128, BPC * FREE], F32, tag="colm")
            nc.sync.dma_start(colm_sb[:], colm[:].partition_broadcast(128))
            for img in range(BPC):
                xt = iop.tile([128, TILE_FREE], F32, tag="xt")
                nc.sync.dma_start(xt[:], x[img])
                yt = iop.tile([128, TILE_FREE], F32, tag="yt")
                for q in range(ROWS_PER_PART):
                    qs = slice(q * FREE, (q + 1) * FREE)
                    nc.vector.scalar_tensor_tensor(
                        yt[:, qs],
                        xt[:, qs],
                        rowm_sb[:, img * ROWS_PER_PART + q : img * ROWS_PER_PART + q + 1],
                        colm_sb[:, img * FREE : (img + 1) * FREE],
                        op0=mult,
                        op1=mult,
                    )
                nc.sync.dma_start(y[img], yt[:])
    nc.compile()
    return nc


def _prep_inputs(x, d_raw, st_h_raw, st_w_raw):
    x = np.ascontiguousarray(np.asarray(x, dtype=np.float32))
    row_keep, col_keep = _build_masks(
        np.asarray(d_raw), np.asarray(st_h_raw), np.asarray(st_w_raw)
    )
    col_exp = np.repeat(col_keep, C, axis=1)  # [B, W*C]
    in_maps = []
    for c in range(NCORES):
        sl = slice(c * BPC, (c + 1) * BPC)
        xc = x[sl].reshape(BPC, 128, TILE_FREE)
        # rowm[p, img*RPP+q] = row_keep[img, RPP*p+q]
        rm = np.ascontiguousarray(
            row_keep[sl]
            .reshape(BPC, 128, ROWS_PER_PART)
            .transpose(1, 0, 2)
            .reshape(128, BPC * ROWS_PER_PART)
        )
        cm = np.ascontiguousarray(col_exp[sl]).reshape(-1)
        in_maps.append({"x": xc, "rowm": rm, "colm": cm})
    return in_maps


def kernel(x, d_raw, st_h_raw, st_w_raw):
    if "nc" not in _CACHE:
        _CACHE["nc"] = _build_nc()
    nc = _CACHE["nc"]
    in_maps = _prep_inputs(x, d_raw, st_h_raw, st_w_raw)
    res = run_bass_kernel_spmd(nc, in_maps, list(range(NCORES)))
    out = np.concatenate(
        [np.asarray(r["y"]).reshape(BPC, H, W, C) for r in res.results], axis=0
    )
    return out


# revision 10
# speedup vs baseline: 1.1298x; 1.0245x over previous
"""GridMask kernel for Trainium2, 8-core data parallel.

out[b,h,w,c] = x[b,h,w,c] * row_keep[b,h] * col_keep[b,w]

The grid mask is separable: a pixel survives iff its row is outside the
horizontal stripes AND its column is outside the vertical stripes. The
tiny per-image row/col keep vectors are computed host-side with exact
integer math; the device kernel streams the 100 MB image tensor through
SBUF applying both mask factors in a single fused
scalar_tensor_tensor op per 128-row tile.

Per core: 4 images, each image split into 4 tiles of [128 rows, 1536
floats] (row-major H x (W*C)). Row mask enters as a per-partition
scalar, col mask as a partition-broadcast [128, 1536] tile loaded once
per image.
"""

import math

import numpy as np

import concourse.bass as bass
import concourse.mybir as mybir
from concourse import bacc, tile
from concourse.bass_utils import run_bass_kernel_spmd

B, H, W, C = 32, 512, 512, 3
D1 = 96
HH = math.ceil(math.sqrt(H * H + W * W))  # 725
OFF_H = (HH - H) // 2  # 106
OFF_W = (HH - W) // 2  # 106

NCORES = 8
BPC = B // NCORES  # images per core
FREE = W * C  # 1536 floats per image row
NBLK = H // 128  # row blocks per image
ROWS = BPC * H  # SBUF-tile rows per core slab

F32 = mybir.dt.float32

_CACHE: dict = {}


def _build_masks(d_raw, st_h_raw, st_w_raw):
    """Exact replica of the reference's integer mask math, in numpy."""
    d = D1 + d_raw.astype(np.int64)  # [B] stripe period
    l = (d + 1) // 2  # ceil(d * 0.5) for integer d
    st_h = st_h_raw.astype(np.int64) % d
    st_w = st_w_raw.astype(np.int64) % d
    yy = OFF_H + np.arange(H, dtype=np.int64)
    xx = OFF_W + np.arange(W, dtype=np.int64)
    row_zero = ((yy[None, :] - st_h[:, None]) % d[:, None]) < l[:, None]
    col_zero = ((xx[None, :] - st_w[:, None]) % d[:, None]) < l[:, None]
    row_keep = (~row_zero).astype(np.float32)  # [B,H]
    col_keep = (~col_zero).astype(np.float32)  # [B,W]
    return row_keep, col_keep


ROWS_PER_PART = H // 128  # 4 consecutive image rows per partition
TILE_FREE = ROWS_PER_PART * FREE  # 6144 floats = 24 KB per partition


def _build_nc():
    nc = bacc.Bacc(None)
    # One tile per image: partition p holds image rows 4p..4p+3, which are
    # contiguous 24 KB in DRAM — big DMA packets instead of 6 KB row packets.
    x = nc.dram_tensor("x", [BPC, 128, TILE_FREE], F32, kind="ExternalInput")
    rowm = nc.dram_tensor("rowm", [128, BPC * ROWS_PER_PART], F32, kind="ExternalInput")
    colm = nc.dram_tensor("colm", [BPC * FREE], F32, kind="ExternalInput")
    y = nc.dram_tensor("y", [BPC, 128, TILE_FREE], F32, kind="ExternalOutput")

    with tile.TileContext(nc) as tc:
        with (
            tc.tile_pool(name="const", bufs=1) as cpool,
            tc.tile_pool(name="io", bufs=2) as iop,
        ):
            # Masks ride the sync HW queue, which is otherwise idle until the
            # first store; image loads go on the scalar (ACT) HW queue.
            rowm_sb = cpool.tile([128, BPC * ROWS_PER_PART], F32, tag="rowm")
            nc.sync.dma_start(rowm_sb[:], rowm[:])
            colm_sb = cpool.tile([128, BPC * FREE], F32, tag="colm")
            nc.sync.dma_start(colm_sb[:], colm[:].partition_broadcast(128))
            for img in range(BPC):
                xt = iop.tile([128, TILE_FREE], F32, tag="xt")
                nc.scalar.dma_start(xt[:], x[img])
                yt = iop.tile([128, TILE_FREE], F32, tag="yt")
                for q in range(ROWS_PER_PART):
                    qs = slice(q * FREE, (q + 1) * FREE)
                    # Row mask on ACT (per-partition scale), col mask on DVE.
                    nc.scalar.mul(
                        yt[:, qs],
                        xt[:, qs],
                        rowm_sb[:, img * ROWS_PER_PART + q : img * ROWS_PER_PART + q + 1],
                    )
                    nc.vector.tensor_mul(
                        yt[:, qs],
                        yt[:, qs],
                        colm_sb[:, img * FREE : (img + 1) * FREE],
                    )
                nc.sync.dma_start(y[img], yt[:])
    nc.compile()
    return nc


def _prep_inputs(x, d_raw, st_h_raw, st_w_raw):
    x = np.ascontiguousarray(np.asarray(x, dtype=np.float32))
    row_keep, col_keep = _build_masks(
        np.asarray(d_raw), np.asarray(st_h_raw), np.asarray(st_w_raw)
    )
    col_exp = np.repeat(col_keep, C, axis=1)  # [B, W*C]
    in_maps = []
    for c in range(NCORES):
        sl = slice(c * BPC, (c + 1) * BPC)
        xc = x[sl].reshape(BPC, 128, TILE_FREE)
        # rowm[p, img*RPP+q] = row_keep[img, RPP*p+q]
        rm = np.ascontiguousarray(
            row_keep[sl]
            .reshape(BPC, 128, ROWS_PER_PART)
            .transpose(1, 0, 2)
            .reshape(128, BPC * ROWS_PER_PART)
        )
        cm = np.ascontiguousarray(col_exp[sl]).reshape(-1)
        in_maps.append({"x": xc, "rowm": rm, "colm": cm})
    return in_maps


def kernel(x, d_raw, st_h_raw, st_w_raw):
    if "nc" not in _CACHE:
        _CACHE["nc"] = _build_nc()
    nc = _CACHE["nc"]
    in_maps = _prep_inputs(x, d_raw, st_h_raw, st_w_raw)
    res = run_bass_kernel_spmd(nc, in_maps, list(range(NCORES)))
    out = np.concatenate(
        [np.asarray(r["y"]).reshape(BPC, H, W, C) for r in res.results], axis=0
    )
    return out


# revision 15
# speedup vs baseline: 1.1527x; 1.0203x over previous
"""GridMask kernel for Trainium2, 8-core data parallel.

out[b,h,w,c] = x[b,h,w,c] * row_keep[b,h] * col_keep[b,w]

The grid mask is separable: a pixel survives iff its row is outside the
horizontal stripes AND its column is outside the vertical stripes. The
tiny per-image row/col keep vectors are computed host-side with exact
integer math; the device kernel streams the 100 MB image tensor through
SBUF applying both mask factors in a single fused
scalar_tensor_tensor op per 128-row tile.

Per core: 4 images, each image split into 4 tiles of [128 rows, 1536
floats] (row-major H x (W*C)). Row mask enters as a per-partition
scalar, col mask as a partition-broadcast [128, 1536] tile loaded once
per image.
"""

import math

import numpy as np

import concourse.bass as bass
import concourse.mybir as mybir
from concourse import bacc, tile
from concourse.bass_utils import run_bass_kernel_spmd

B, H, W, C = 32, 512, 512, 3
D1 = 96
HH = math.ceil(math.sqrt(H * H + W * W))  # 725
OFF_H = (HH - H) // 2  # 106
OFF_W = (HH - W) // 2  # 106

NCORES = 8
BPC = B // NCORES  # images per core
FREE = W * C  # 1536 floats per image row
NBLK = H // 128  # row blocks per image
ROWS = BPC * H  # SBUF-tile rows per core slab

F32 = mybir.dt.float32

_CACHE: dict = {}


def _build_masks(d_raw, st_h_raw, st_w_raw):
    """Exact replica of the reference's integer mask math, in numpy."""
    d = D1 + d_raw.astype(np.int64)  # [B] stripe period
    l = (d + 1) // 2  # ceil(d * 0.5) for integer d
    st_h = st_h_raw.astype(np.int64) % d
    st_w = st_w_raw.astype(np.int64) % d
    yy = OFF_H + np.arange(H, dtype=np.int64)
    xx = OFF_W + np.arange(W, dtype=np.int64)
    row_zero = ((yy[None, :] - st_h[:, None]) % d[:, None]) < l[:, None]
    col_zero = ((xx[None, :] - st_w[:, None]) % d[:, None]) < l[:, None]
    row_keep = (~row_zero).astype(np.float32)  # [B,H]
    col_keep = (~col_zero).astype(np.float32)  # [B,W]
    return row_keep, col_keep


IMGS_PER_TILE = 2  # two images per SBUF tile
NTILES = BPC // IMGS_PER_TILE  # 2 tiles per core
RPP = IMGS_PER_TILE * H // 128  # 8 consecutive image rows per partition
TILE_FREE = RPP * FREE  # 12288 floats = 48 KB per partition
PPI = 128 // IMGS_PER_TILE  # partitions per image within a tile (64)


def _build_nc():
    nc = bacc.Bacc(None)
    # Two images per tile: partition p holds rows 8p..8p+7 of the 2-image
    # slab — 48 KB contiguous in DRAM per partition, so each DMA moves
    # 48 KB packets (DMA engines are overhead-bound at smaller packets).
    # Partitions 0..63 hold the even image, 64..127 the odd image.
    x = nc.dram_tensor("x", [NTILES, 128, TILE_FREE], F32, kind="ExternalInput")
    rowm = nc.dram_tensor("rowm", [128, NTILES * RPP], F32, kind="ExternalInput")
    colm = nc.dram_tensor("colm", [128, NTILES * FREE], F32, kind="ExternalInput")
    y = nc.dram_tensor("y", [NTILES, 128, TILE_FREE], F32, kind="ExternalOutput")

    mult = mybir.AluOpType.mult
    with tile.TileContext(nc) as tc:
        with (
            tc.tile_pool(name="const", bufs=1) as cpool,
            tc.tile_pool(name="io", bufs=3) as iop,
        ):
            # Masks ride the sync HW queue, which is otherwise idle until the
            # first store; image loads go on the scalar (ACT) HW queue.
            rowm_sb = cpool.tile([128, NTILES * RPP], F32, tag="rowm")
            nc.sync.dma_start(rowm_sb[:], rowm[:])
            # colm dram is prearranged host-side as [128, NTILES*FREE]:
            # partitions 0..63 carry the even image's col mask, 64..127 the
            # odd image's, so a plain 2-D DMA loads each tile's mask.
            colm_sbs = []
            for t in range(NTILES):
                cm = cpool.tile([128, FREE], F32, tag=f"colm{t}")
                nc.sync.dma_start(cm[:], colm[:, t * FREE : (t + 1) * FREE])
                colm_sbs.append(cm)
            for t in range(NTILES):
                xt = iop.tile([128, TILE_FREE], F32, tag="xt")
                nc.scalar.dma_start(xt[:], x[t])
                for r in range(RPP):
                    rs = slice(r * FREE, (r + 1) * FREE)
                    nc.vector.scalar_tensor_tensor(
                        xt[:, rs],
                        xt[:, rs],
                        rowm_sb[:, t * RPP + r : t * RPP + r + 1],
                        colm_sbs[t][:],
                        op0=mult,
                        op1=mult,
                    )
                nc.sync.dma_start(y[t], xt[:])
    nc.compile()
    return nc


def _prep_inputs(x, d_raw, st_h_raw, st_w_raw):
    x = np.ascontiguousarray(np.asarray(x, dtype=np.float32))
    row_keep, col_keep = _build_masks(
        np.asarray(d_raw), np.asarray(st_h_raw), np.asarray(st_w_raw)
    )
    col_exp = np.repeat(col_keep, C, axis=1)  # [B, W*C]
    in_maps = []
    for c in range(NCORES):
        sl = slice(c * BPC, (c + 1) * BPC)
        xc = x[sl].reshape(NTILES, 128, TILE_FREE)
        # rowm[p, t*RPP+r] = keep of global row 8p+r within tile t's 2-image slab
        rm = np.ascontiguousarray(
            row_keep[sl]
            .reshape(NTILES, 128, RPP)
            .transpose(1, 0, 2)
            .reshape(128, NTILES * RPP)
        )
        # colm[p, t*FREE + f] = col mask of the image in partition p of tile t
        ce = col_exp[sl].reshape(NTILES, IMGS_PER_TILE, FREE)  # [t, h, f]
        cm = np.ascontiguousarray(
            np.repeat(ce, PPI, axis=1)  # [t, 128, f]
            .transpose(1, 0, 2)
            .reshape(128, NTILES * FREE)
        )
        in_maps.append({"x": xc, "rowm": rm, "colm": cm})
    return in_maps


def kernel(x, d_raw, st_h_raw, st_w_raw):
    if "nc" not in _CACHE:
        _CACHE["nc"] = _build_nc()
    nc = _CACHE["nc"]
    in_maps = _prep_inputs(x, d_raw, st_h_raw, st_w_raw)
    res = run_bass_kernel_spmd(nc, in_maps, list(range(NCORES)))
    out = np.concatenate(
        [np.asarray(r["y"]).reshape(BPC, H, W, C) for r in res.results], axis=0
    )
    return out


# revision 19
# speedup vs baseline: 1.1836x; 1.0269x over previous
"""GridMask kernel for Trainium2, 8-core data parallel.

out[b,h,w,c] = x[b,h,w,c] * row_keep[b,h] * col_keep[b,w]

The grid mask is separable: a pixel survives iff its row is outside the
horizontal stripes AND its column is outside the vertical stripes. The
tiny per-image row/col keep vectors are computed host-side with exact
integer math; the device kernel streams the 100 MB image tensor through
SBUF applying both mask factors in a single fused
scalar_tensor_tensor op per 128-row tile.

Per core: 4 images, each image split into 4 tiles of [128 rows, 1536
floats] (row-major H x (W*C)). Row mask enters as a per-partition
scalar, col mask as a partition-broadcast [128, 1536] tile loaded once
per image.
"""

import math

import ml_dtypes
import numpy as np

import concourse.bass as bass
import concourse.mybir as mybir
from concourse import bacc, tile
from concourse.bass_utils import run_bass_kernel_spmd

B, H, W, C = 32, 512, 512, 3
D1 = 96
HH = math.ceil(math.sqrt(H * H + W * W))  # 725
OFF_H = (HH - H) // 2  # 106
OFF_W = (HH - W) // 2  # 106

NCORES = 8
BPC = B // NCORES  # images per core
FREE = W * C  # 1536 floats per image row
NBLK = H // 128  # row blocks per image
ROWS = BPC * H  # SBUF-tile rows per core slab

F32 = mybir.dt.float32

_CACHE: dict = {}


def _build_masks(d_raw, st_h_raw, st_w_raw):
    """Exact replica of the reference's integer mask math, in numpy."""
    d = D1 + d_raw.astype(np.int64)  # [B] stripe period
    l = (d + 1) // 2  # ceil(d * 0.5) for integer d
    st_h = st_h_raw.astype(np.int64) % d
    st_w = st_w_raw.astype(np.int64) % d
    yy = OFF_H + np.arange(H, dtype=np.int64)
    xx = OFF_W + np.arange(W, dtype=np.int64)
    row_zero = ((yy[None, :] - st_h[:, None]) % d[:, None]) < l[:, None]
    col_zero = ((xx[None, :] - st_w[:, None]) % d[:, None]) < l[:, None]
    row_keep = (~row_zero).astype(np.float32)  # [B,H]
    col_keep = (~col_zero).astype(np.float32)  # [B,W]
    return row_keep, col_keep


BF16 = mybir.dt.bfloat16
NTILES = BPC  # one image per tile
RPP = H // 128  # 4 consecutive image rows per partition
TILE_FREE = RPP * FREE  # 6144 floats = 24 KB per partition


def _build_nc():
    nc = bacc.Bacc(None)
    # One image per tile: partition p holds image rows 4p..4p+3 — 24 KB
    # contiguous in DRAM per partition (the packet size where the DMA
    # engines hit their best per-engine rate).
    x = nc.dram_tensor("x", [NTILES, 128, TILE_FREE], F32, kind="ExternalInput")
    rowm = nc.dram_tensor("rowm", [128, NTILES * RPP], F32, kind="ExternalInput")
    # col masks are 0/1 so bf16 is exact — halves the broadcast traffic
    colm = nc.dram_tensor("colm", [128, NTILES * FREE], F32, kind="ExternalInput")
    y = nc.dram_tensor("y", [NTILES, 128, TILE_FREE], F32, kind="ExternalOutput")

    mult = mybir.AluOpType.mult
    with tile.TileContext(nc) as tc:
        with (
            tc.tile_pool(name="const", bufs=1) as cpool,
            tc.tile_pool(name="io", bufs=6) as iop,
        ):
            # Masks go on the gpsimd SWDGE queue, keeping both HW queues
            # free for the bulk load (scalar/ACT queue) and store (sync).
            rowm_sb = cpool.tile([128, NTILES * RPP], F32, tag="rowm")
            nc.gpsimd.dma_start(rowm_sb[:], rowm[:])
            colm_sb = cpool.tile([128, NTILES * FREE], F32, tag="colm")
            nc.gpsimd.dma_start(colm_sb[:], colm[:])
            for t in range(NTILES):
                xt = iop.tile([128, TILE_FREE], F32, tag="xt")
                nc.scalar.dma_start(xt[:], x[t])
                for r in range(RPP):
                    rs = slice(r * FREE, (r + 1) * FREE)
                    nc.vector.scalar_tensor_tensor(
                        xt[:, rs],
                        xt[:, rs],
                        rowm_sb[:, t * RPP + r : t * RPP + r + 1],
                        colm_sb[:, t * FREE : (t + 1) * FREE],
                        op0=mult,
                        op1=mult,
                    )
                nc.sync.dma_start(y[t], xt[:])
    nc.compile()
    return nc


def _prep_inputs(x, d_raw, st_h_raw, st_w_raw):
    x = np.ascontiguousarray(np.asarray(x, dtype=np.float32))
    row_keep, col_keep = _build_masks(
        np.asarray(d_raw), np.asarray(st_h_raw), np.asarray(st_w_raw)
    )
    col_exp = np.repeat(col_keep, C, axis=1)  # [B, W*C]
    in_maps = []
    for c in range(NCORES):
        sl = slice(c * BPC, (c + 1) * BPC)
        xc = x[sl].reshape(NTILES, 128, TILE_FREE)
        # rowm[p, t*RPP+r] = keep of image row 4p+r of image t
        rm = np.ascontiguousarray(
            row_keep[sl]
            .reshape(NTILES, 128, RPP)
            .transpose(1, 0, 2)
            .reshape(128, NTILES * RPP)
        )
        # colm[p, t*FREE + f] = col mask of image t (same for all partitions)
        cm = np.ascontiguousarray(
            np.broadcast_to(
                col_exp[sl].reshape(1, NTILES * FREE), (128, NTILES * FREE)
            )
        ).astype(np.float32)
        in_maps.append({"x": xc, "rowm": rm, "colm": cm})
    return in_maps


def kernel(x, d_raw, st_h_raw, st_w_raw):
    if "nc" not in _CACHE:
        _CACHE["nc"] = _build_nc()
    nc = _CACHE["nc"]
    in_maps = _prep_inputs(x, d_raw, st_h_raw, st_w_raw)
    res = run_bass_kernel_spmd(nc, in_maps, list(range(NCORES)))
    out = np.concatenate(
        [np.asarray(r["y"]).reshape(BPC, H, W, C) for r in res.results], axis=0
    )
    return out
